# revision 1
# baseline (speedup 1.0000x reference)
import os
import sys

for _p in ("/opt/trn_rl_repo", "/root/.axon_site/_ro/trn_rl_repo"):
    if os.path.isdir(_p) and _p not in sys.path:
        sys.path.insert(0, _p)

import numpy as np
import ml_dtypes

BF16 = ml_dtypes.bfloat16

HEADS, D = 12, 64
WINDOW, SHIFT = 16, 1
SCALE = D ** -0.5
B, N, DIM = 2, 2049, 768
INNER = HEADS * D  # 768
TAUG = 258  # CLS slot + tok1/dummy slot + 256 block tokens
NCORES = 8
KT = DIM // 128  # 6
VW = HEADS * 65  # 780: per-head 64 v-cols + ones-col at 65h+64

STARTS = [2, 258, 514, 770, 1026, 1282, 1538, 1794]
ENDS = [258, 514, 770, 1026, 1282, 1538, 1794, 2049]

_NC_CACHE = {}


def _build_nc():
    import concourse.bass as bass
    import concourse.bacc as bacc
    import concourse.mybir as mybir
    import concourse.tile as tile

    f32 = mybir.dt.float32
    bf16 = mybir.dt.bfloat16
    Exp = mybir.ActivationFunctionType.Exp
    Copy = mybir.ActivationFunctionType.Copy

    nc = bacc.Bacc(None, target_bir_lowering=False)

    xT_ext = nc.declare_dram_parameter("xaT", (B, KT, 128, TAUG), bf16, isOutput=False)
    wqkv_ext = nc.declare_dram_parameter("w_qkv", (DIM, 1536 + VW), bf16, isOutput=False)
    wout_ext = nc.declare_dram_parameter("w_out", (INNER, DIM), bf16, isOutput=False)
    bout_ext = nc.declare_dram_parameter("b_out", (128, DIM), f32, isOutput=False)
    mask_ext = nc.declare_dram_parameter("masks", (2, 128, 128), bf16, isOutput=False)
    id_ext = nc.declare_dram_parameter("ident", (128, 128), bf16, isOutput=False)
    vcr_ext = nc.declare_dram_parameter("vc_rep", (B, 2, VW), bf16, isOutput=False)
    out_ext = nc.declare_dram_parameter("out_tokens", (B, 256, DIM), f32, isOutput=True)
    cls_ext = nc.declare_dram_parameter("cls_part", (B, HEADS, VW), f32, isOutput=True)
    t1x_ext = nc.declare_dram_parameter("t1x_part", (B, 2 * HEADS, VW), f32, isOutput=True)

    with tile.TileContext(nc) as tc:
        with (
            tc.tile_pool(name="wpool", bufs=1) as wp,
            tc.tile_pool(name="fpool", bufs=2) as fp,
            tc.tile_pool(name="spool", bufs=6) as sp,
            tc.tile_pool(name="psA", bufs=2, space="PSUM") as psA,
            tc.tile_pool(name="psP", bufs=2, space="PSUM") as psP,
            tc.tile_pool(name="psS", bufs=3, space="PSUM") as psS,
        ):
            # ---- input DMAs first (x before weights so compute starts early) ----
            xT, vcr = [], []
            for b in range(B):
                row = []
                for k in range(KT):
                    t = fp.tile([128, TAUG], bf16, tag=f"xT{b}_{k}", name=f"xT{b}_{k}")
                    nc.sync.dma_start(t[:], xT_ext[b, k])
                    row.append(t)
                xT.append(row)
                vt = fp.tile([2, VW], bf16, tag=f"vcr{b}", name=f"vcr{b}")
                nc.sync.dma_start(vt[:], vcr_ext[b])
                vcr.append(vt)
            ident = wp.tile([128, 128], bf16, tag="ident")
            nc.sync.dma_start(ident[:], id_ext[:])
            mask_t = []
            for s in range(2):
                m = wp.tile([128, 128], bf16, tag=f"mask{s}")
                nc.sync.dma_start(m[:], mask_ext[s])
                mask_t.append(m)
            w_tiles = []
            for k in range(KT):
                t = wp.tile([128, 1536 + VW], bf16, tag=f"wqkv{k}")
                nc.sync.dma_start(t[:], wqkv_ext[k * 128:(k + 1) * 128, :])
                w_tiles.append(t)
            wo_t = []
            for k in range(KT):
                t = wp.tile([128, DIM], bf16, tag=f"wo{k}")
                nc.sync.dma_start(t[:], wout_ext[k * 128:(k + 1) * 128, :])
                wo_t.append(t)
            bias_full = wp.tile([128, DIM], f32, tag="bias_full")
            nc.sync.dma_start(bias_full[:], bout_ext[:])

            qT = [[None] * (HEADS // 2) for _ in range(B)]
            kTt = [[None] * (HEADS // 2) for _ in range(B)]
            vs = [[None, None] for _ in range(B)]
            att_s = [[fp.tile([128, INNER], bf16, tag=f"att{b}_{s}", name=f"att{b}_{s}")
                      for s in range(2)] for b in range(B)]
            aT = [[fp.tile([128, 256], bf16, tag=f"aT{b}_{i}", name=f"aT{b}_{i}")
                   for i in range(KT)] for b in range(B)]
            clspt = psS.tile([128, 512], f32, tag="cls", bufs=1, name="clspt")
            clsp = [clspt[:, 256 * b:256 * b + 256] for b in range(B)]

            def qk_unit(b, base, pref, g, dst):
                ps = psA.tile([128, 512], f32, tag="big", name="ps")
                for k in range(KT):
                    nc.tensor.matmul(ps[:, 0:TAUG],
                                     w_tiles[k][:, base + 128 * g: base + 128 * (g + 1)],
                                     xT[b][k][:], start=(k == 0), stop=(k == KT - 1))
                t = fp.tile([128, TAUG], bf16, tag=f"{pref}T{b}_{g}", name=f"{pref}T{b}_{g}")
                nc.vector.tensor_copy(t[:], ps[:, 0:TAUG])
                dst[b][g] = t

            def v_unit(b, ti):
                vt = fp.tile([128, VW], bf16, tag=f"v{b}_{ti}", name=f"v{b}_{ti}")
                for c0, cw in ((0, 512), (512, VW - 512)):
                    pv = psA.tile([128, 512], f32, tag="big", name="pv")
                    for k in range(KT):
                        nc.tensor.matmul(pv[:, 0:cw],
                                         xT[b][k][:, 2 + 128 * ti: 2 + 128 * (ti + 1)],
                                         w_tiles[k][:, 1536 + c0: 1536 + c0 + cw],
                                         start=(k == 0), stop=(k == KT - 1))
                    nc.vector.tensor_copy(vt[:, c0:c0 + cw], pv[:, 0:cw])
                nc.vector.memset(vt[:, 64:VW:65], 1.0)
                vs[b][ti] = vt

            def attn_head(b, h):
                g, p0 = h // 2, 64 * (h % 2)
                kk, qq = kTt[b][g], qT[b][g]
                hps = psS.tile([128, 512], f32, tag="hps", name="hps")
                nc.tensor.matmul(hps[0:1, 0:TAUG], kk[p0:p0 + 64, 0:1], qq[p0:p0 + 64, :],
                                 start=True, stop=True, skip_group_check=True)
                ecr = sp.tile([1, TAUG], bf16, tag="ecr", name="ecr")
                nc.scalar.activation(ecr[:], hps[0:1, 0:TAUG], Exp, scale=SCALE)
                for s in range(2):
                    q0 = 2 + 128 * s
                    nc.tensor.matmul(clsp[b][:, 12 * s + h:12 * s + h + 1],
                                     kk[p0:p0 + 64, q0:q0 + 128],
                                     qq[p0:p0 + 64, 0:1], start=True, stop=True,
                                     skip_group_check=True)
                nc.tensor.matmul(clsp[b][0:2, 24 + 2 * h:24 + 2 * h + 2],
                                 kk[p0:p0 + 64, 0:2],
                                 qq[p0:p0 + 64, 0:2], start=True, stop=True,
                                 skip_group_check=True)
                for s in range(2):
                    q0 = 2 + 128 * s
                    pc = TAUG + 65 * s
                    pst = psP.tile([128, 128], f32, tag="pq", name="pst")
                    nc.tensor.matmul(pst[:], kk[p0:p0 + 64, q0:q0 + 128],
                                     qq[p0:p0 + 64, q0:q0 + 128], start=True, stop=True)
                    prob = sp.tile([128, 128], bf16, tag="prob", name="prob")
                    nc.scalar.activation(prob[:], pst[:], Exp, scale=SCALE)
                    nc.vector.tensor_mul(prob[:], prob[:], mask_t[s][:])
                    nc.tensor.matmul(hps[:, pc:pc + 65], prob[:],
                                     vs[b][s][:, 65 * h:65 * h + 65],
                                     start=True, stop=False, skip_group_check=True)
                    nc.tensor.matmul(hps[:, pc:pc + 65], ecr[:, q0:q0 + 128],
                                     vcr[b][0:1, 65 * h:65 * h + 65],
                                     start=False, stop=True, skip_group_check=True)
                    rec = sp.tile([128, 1], f32, tag="rec", name="rec")
                    nc.vector.reciprocal(rec[:], hps[:, pc + 64:pc + 65])
                    nc.scalar.activation(att_s[b][s][:, 64 * h:64 * h + 64],
                                         hps[:, pc:pc + 64], Copy, scale=rec[:, 0:1])

            def cls_unit(b):
                eccs = []
                for s in range(2):
                    E = sp.tile([128, HEADS], bf16, tag="ECC", name="E")
                    nc.scalar.activation(E[:], clsp[b][:, 12 * s:12 * (s + 1)], Exp, scale=SCALE)
                    eccs.append(E)
                clsA = psS.tile([128, 512], f32, tag="hps", name="clsA")
                clsB = psS.tile([128, 512], f32, tag="hps", name="clsB")
                for s in range(2):
                    nc.tensor.matmul(clsA[0:HEADS, :], eccs[s][:], vs[b][s][:, 0:512],
                                     start=(s == 0), stop=(s == 1), skip_group_check=True)
                    nc.tensor.matmul(clsB[0:HEADS, 0:VW - 512], eccs[s][:], vs[b][s][:, 512:VW],
                                     start=(s == 0), stop=(s == 1), skip_group_check=True)
                cls_sb = sp.tile([HEADS, VW], f32, tag="clssb", name="cls_sb")
                nc.vector.tensor_copy(cls_sb[:, 0:512], clsA[0:HEADS, :])
                nc.vector.tensor_copy(cls_sb[:, 512:VW], clsB[0:HEADS, 0:VW - 512])
                nc.sync.dma_start(cls_ext[b], cls_sb[:])
                ET1 = sp.tile([2, 2 * HEADS], bf16, tag="ET1", name="ET1")
                nc.scalar.activation(ET1[:], clsp[b][0:2, 24:24 + 2 * HEADS], Exp, scale=SCALE)
                t1A = psS.tile([128, 512], f32, tag="hps", name="t1A")
                t1B = psS.tile([128, 512], f32, tag="hps", name="t1B")
                nc.tensor.matmul(t1A[0:24, :], ET1[:], vcr[b][0:2, 0:512], start=True, stop=True,
                                 skip_group_check=True)
                nc.tensor.matmul(t1B[0:24, 0:VW - 512], ET1[:], vcr[b][0:2, 512:VW], start=True,
                                 stop=True, skip_group_check=True)
                t1_sb = sp.tile([24, VW], f32, tag="t1sb", name="t1_sb")
                nc.vector.tensor_copy(t1_sb[:, 0:512], t1A[0:24, :])
                nc.vector.tensor_copy(t1_sb[:, 512:VW], t1B[0:24, 0:VW - 512])
                nc.sync.dma_start(t1x_ext[b], t1_sb[:])

            def tr_unit(b, i):
                for s in range(2):
                    pt = psA.tile([128, 128], bf16, tag="big", name="pt")
                    nc.tensor.transpose(pt[:], att_s[b][s][:, 128 * i:128 * (i + 1)], ident[:])
                    nc.vector.tensor_copy(aT[b][i][:, 128 * s:128 * (s + 1)], pt[:])

            def proj_unit(b, ti):
                ot = fp.tile([128, DIM], f32, tag=f"ot{b}_{ti}", name=f"ot{b}_{ti}")
                for c0, cw in ((0, 512), (512, 256)):
                    po = psA.tile([128, 512], f32, tag="big", name="po")
                    for i in range(KT):
                        nc.tensor.matmul(po[:, 0:cw], aT[b][i][:, 128 * ti:128 * (ti + 1)],
                                         wo_t[i][:, c0:c0 + cw],
                                         start=(i == 0), stop=(i == KT - 1))
                    nc.vector.tensor_add(ot[:, c0:c0 + cw], po[:, 0:cw],
                                         bias_full[:, c0:c0 + cw])
                nc.sync.dma_start(out_ext[b, 128 * ti:128 * (ti + 1), :], ot[:])

            # ---- phase 1: QKV(b0) dense ----
            for g in range(HEADS // 2):
                qk_unit(0, 0, "q", g, qT)
                qk_unit(0, INNER, "k", g, kTt)
            v_unit(0, 0)
            v_unit(0, 1)

            # ---- phase 2: attention(b0) interleaved with QKV(b1) ----
            b1_units = ([lambda g=g: qk_unit(1, 0, "q", g, qT) for g in range(HEADS // 2)]
                        + [lambda g=g: qk_unit(1, INNER, "k", g, kTt) for g in range(HEADS // 2)]
                        + [lambda ti=ti: v_unit(1, ti) for ti in range(2)])
            for h in range(HEADS):
                attn_head(0, h)
                u = b1_units[7 * h // 6:7 * (h + 1) // 6]
                for f in u:
                    f()
            for f in b1_units[14:]:
                f()

            # ---- phase 3: attention(b1) interleaved with cls/transpose/proj(b0) ----
            b0_tail = ([lambda: cls_unit(0)]
                       + [lambda i=i: tr_unit(0, i) for i in range(KT)]
                       + [lambda ti=ti: proj_unit(0, ti) for ti in range(2)])
            for h in range(HEADS):
                attn_head(1, h)
                u = b0_tail[9 * h // 12:9 * (h + 1) // 12]
                for f in u:
                    f()
            for f in b0_tail[9:]:
                f()

            # ---- phase 4: tail for b1 ----
            cls_unit(1)
            for i in range(KT):
                tr_unit(1, i)
            for ti in range(2):
                proj_unit(1, ti)

    nc.compile()
    return nc


def _get_nc():
    if "nc" not in _NC_CACHE:
        _NC_CACHE["nc"] = _build_nc()
    return _NC_CACHE["nc"]


def _make_masks(core):
    start = STARTS[core]
    masks = np.zeros((2, 128, 128), dtype=np.float32)
    for s in range(2):
        g = start + s * 128 + np.arange(128)
        real = g < 2049
        blk = (g - 2) // 16
        same = (blk[:, None] == blk[None, :]) & real[:, None] & real[None, :]
        masks[s] = same.astype(np.float32)
    return masks.astype(BF16)


def _make_in_maps(x, w_qkv, w_out, b_out):
    x = np.asarray(x, dtype=np.float32)
    w_qkv = np.asarray(w_qkv, dtype=np.float32)
    w_out_b = np.asarray(w_out, dtype=np.float32).astype(BF16)
    b_out = np.asarray(b_out, dtype=np.float32)

    w_dev = np.zeros((DIM, 1536 + VW), dtype=np.float32)
    w_dev[:, 0:1536] = w_qkv[:, 0:1536]
    for h in range(HEADS):
        w_dev[:, 1536 + 65 * h:1536 + 65 * h + 64] = w_qkv[:, 1536 + 64 * h:1536 + 64 * h + 64]
    w_qkv_b = w_dev.astype(BF16)

    w_v = w_qkv[:, 1536:]
    vcls = x[:, 0, :] @ w_v
    vtok1 = x[:, 1, :] @ w_v

    def v65(row768):
        out = np.zeros(VW, dtype=np.float32)
        for h in range(HEADS):
            out[65 * h:65 * h + 64] = row768[64 * h:64 * h + 64]
            out[65 * h + 64] = 1.0
        return out

    ident = np.eye(128, dtype=BF16)
    bias_tiled = np.tile(b_out.reshape(1, DIM), (128, 1)).astype(np.float32)
    in_maps = []
    for c in range(NCORES):
        xa = np.zeros((B, TAUG, DIM), dtype=np.float32)
        xa[:, 0, :] = x[:, 0, :]
        if c == 0:
            xa[:, 1, :] = x[:, 1, :]
        L = ENDS[c] - STARTS[c]
        xa[:, 2:2 + L, :] = x[:, STARTS[c]:ENDS[c], :]
        xaT = xa.transpose(0, 2, 1).reshape(B, KT, 128, TAUG).astype(BF16)
        vcr = np.zeros((B, 2, VW), dtype=np.float32)
        for b in range(B):
            vcr[b, 0] = v65(vcls[b])
            vcr[b, 1] = v65(vtok1[b] if c == 0 else np.zeros(INNER, np.float32))
        in_maps.append({
            "vc_rep": vcr.astype(BF16),
            "xaT": xaT,
            "w_qkv": w_qkv_b,
            "w_out": w_out_b,
            "b_out": bias_tiled,
            "masks": _make_masks(c),
            "ident": ident,
        })
    return in_maps


def kernel(x, w_qkv, w_out, b_out):
    w_out_f = np.asarray(w_out, dtype=np.float32)
    b_out_f = np.asarray(b_out, dtype=np.float32)
    in_maps = _make_in_maps(x, w_qkv, w_out, b_out)

    from concourse.bass_utils import run_bass_kernel_spmd

    nc = _get_nc()
    res = run_bass_kernel_spmd(nc, in_maps, core_ids=list(range(NCORES))).results

    out = np.empty((B, N, DIM), dtype=np.float32)
    for c in range(NCORES):
        L = ENDS[c] - STARTS[c]
        out[:, STARTS[c]:ENDS[c], :] = res[c]["out_tokens"][:, :L, :]

    for b in range(B):
        acc = np.zeros((HEADS, VW), dtype=np.float64)
        for c in range(NCORES):
            acc += res[c]["cls_part"][b].astype(np.float64)
            acc[:, 64::65] -= 256 - (ENDS[c] - STARTS[c])
        t1x = res[0]["t1x_part"][b].astype(np.float64)
        acc += t1x[0::2]
        cls_flat = np.empty(INNER, dtype=np.float64)
        t1_flat = np.empty(INNER, dtype=np.float64)
        for h in range(HEADS):
            cls_flat[64 * h:64 * h + 64] = acc[h, 65 * h:65 * h + 64] / acc[h, 65 * h + 64]
            t1_flat[64 * h:64 * h + 64] = (t1x[2 * h + 1, 65 * h:65 * h + 64]
                                           / t1x[2 * h + 1, 65 * h + 64])
        out[b, 0, :] = (cls_flat @ w_out_f + b_out_f).astype(np.float32)
        out[b, 1, :] = (t1_flat @ w_out_f + b_out_f).astype(np.float32)
    return out



# revision 9
# speedup vs baseline: 1.2977x; 1.2977x over previous
import os
import sys

for _p in ("/opt/trn_rl_repo", "/root/.axon_site/_ro/trn_rl_repo"):
    if os.path.isdir(_p) and _p not in sys.path:
        sys.path.insert(0, _p)

import numpy as np
import ml_dtypes

BF16 = ml_dtypes.bfloat16

HEADS, D = 12, 64
WINDOW, SHIFT = 16, 1
SCALE = D ** -0.5
B, N, DIM = 2, 2049, 768
INNER = HEADS * D  # 768
TAUG = 258  # CLS slot + tok1/dummy slot + 256 block tokens
NCORES = 8
KT = DIM // 128  # 6
VW = HEADS * 65  # 780: per-head 64 v-cols + ones-col at 65h+64

STARTS = [2, 258, 514, 770, 1026, 1282, 1538, 1794]
ENDS = [258, 514, 770, 1026, 1282, 1538, 1794, 2049]

_NC_CACHE = {}


def _build_nc():
    import concourse.bass as bass
    import concourse.bacc as bacc
    import concourse.mybir as mybir
    import concourse.tile as tile

    f32 = mybir.dt.float32
    bf16 = mybir.dt.bfloat16
    Exp = mybir.ActivationFunctionType.Exp
    Copy = mybir.ActivationFunctionType.Copy

    nc = bacc.Bacc(None, target_bir_lowering=False)

    xT_ext = nc.declare_dram_parameter("xaT", (B, KT, 128, TAUG), bf16, isOutput=False)
    # w_qkv for q/k repacked on host into 12 column-slices (q0,k0,q1,k1,...)
    # of shape [128, KT*128] so each slice is one DMA and compute on slice s
    # only depends on that slice's DMA (compute starts ~2us in).
    wqk_ext = nc.declare_dram_parameter("wqk", (12, 128, KT * 128), bf16, isOutput=False)
    wv_ext = nc.declare_dram_parameter("wv", (KT, 128, VW), bf16, isOutput=False)
    wout_ext = nc.declare_dram_parameter("w_out", (INNER, DIM), bf16, isOutput=False)
    bout_ext = nc.declare_dram_parameter("b_out", (128, DIM), bf16, isOutput=False)
    mask_ext = nc.declare_dram_parameter("masks", (2, 128, 128), bf16, isOutput=False)
    id_ext = nc.declare_dram_parameter("ident", (128, 128), bf16, isOutput=False)
    vcr_ext = nc.declare_dram_parameter("vc_rep", (B, 2, VW), bf16, isOutput=False)
    out_ext = nc.declare_dram_parameter("out_tokens", (B, 256, DIM), bf16, isOutput=True)
    cls_ext = nc.declare_dram_parameter("cls_part", (B, HEADS, VW), f32, isOutput=True)
    t1x_ext = nc.declare_dram_parameter("t1x_part", (B, 2 * HEADS, VW), f32, isOutput=True)

    with tile.TileContext(nc) as tc:
        with (
            tc.tile_pool(name="wpool", bufs=1) as wp,
            tc.tile_pool(name="fpool", bufs=2) as fp,
            tc.tile_pool(name="spool", bufs=6) as sp,
            tc.tile_pool(name="psA", bufs=2, space="PSUM") as psA,
            tc.tile_pool(name="psP", bufs=2, space="PSUM") as psP,
            tc.tile_pool(name="psS", bufs=3, space="PSUM") as psS,
        ):
            # ---- input DMAs in compute order: x(b0), qk weight slices, v
            # weights, masks, x(b1), then late-needed w_out/bias ----
            xT = [[None] * KT for _ in range(B)]
            vcr = [None] * B
            for k in range(KT):
                t = fp.tile([128, TAUG], bf16, tag=f"xT0_{k}", name=f"xT0_{k}")
                nc.sync.dma_start(t[:], xT_ext[0, k])
                xT[0][k] = t
            vcr[0] = fp.tile([2, VW], bf16, tag="vcr0", name="vcr0")
            nc.sync.dma_start(vcr[0][:], vcr_ext[0])
            wqk_t = []
            for s in range(12):
                t = wp.tile([128, KT * 128], bf16, tag=f"wqk{s}")
                nc.sync.dma_start(t[:], wqk_ext[s])
                wqk_t.append(t)
            wv_t = []
            for k in range(KT):
                t = wp.tile([128, VW], bf16, tag=f"wv{k}")
                nc.sync.dma_start(t[:], wv_ext[k])
                wv_t.append(t)
            ident = wp.tile([128, 128], bf16, tag="ident")
            nc.sync.dma_start(ident[:], id_ext[:])
            mask_t = []
            for s in range(2):
                m = wp.tile([128, 128], bf16, tag=f"mask{s}")
                nc.sync.dma_start(m[:], mask_ext[s])
                mask_t.append(m)
            for k in range(KT):
                t = fp.tile([128, TAUG], bf16, tag=f"xT1_{k}", name=f"xT1_{k}")
                nc.sync.dma_start(t[:], xT_ext[1, k])
                xT[1][k] = t
            vcr[1] = fp.tile([2, VW], bf16, tag="vcr1", name="vcr1")
            nc.sync.dma_start(vcr[1][:], vcr_ext[1])
            wo_t = []
            for k in range(KT):
                t = wp.tile([128, DIM], bf16, tag=f"wo{k}")
                nc.sync.dma_start(t[:], wout_ext[k * 128:(k + 1) * 128, :])
                wo_t.append(t)
            bias_full = wp.tile([128, DIM], bf16, tag="bias_full")
            nc.sync.dma_start(bias_full[:], bout_ext[:])

            qT = [[None] * (HEADS // 2) for _ in range(B)]
            kTt = [[None] * (HEADS // 2) for _ in range(B)]
            vs = [[None, None] for _ in range(B)]
            att_s = [[fp.tile([128, INNER], bf16, tag=f"att{b}_{s}", name=f"att{b}_{s}")
                      for s in range(2)] for b in range(B)]
            aT = [[fp.tile([128, 256], bf16, tag=f"aT{b}_{i}", name=f"aT{b}_{i}")
                   for i in range(KT)] for b in range(B)]
            clspt = psS.tile([128, 512], f32, tag="cls", bufs=1, name="clspt")
            clsp = [clspt[:, 256 * b:256 * b + 256] for b in range(B)]

            def qk_unit(b, slice_idx, pref, g, dst):
                w = wqk_t[slice_idx]
                ps = psA.tile([128, 512], f32, tag="big", name="ps")
                for k in range(KT):
                    nc.tensor.matmul(ps[:, 0:TAUG],
                                     w[:, 128 * k: 128 * (k + 1)],
                                     xT[b][k][:], start=(k == 0), stop=(k == KT - 1))
                t = fp.tile([128, TAUG], bf16, tag=f"{pref}T{b}_{g}", name=f"{pref}T{b}_{g}")
                nc.vector.tensor_copy(t[:], ps[:, 0:TAUG])
                dst[b][g] = t

            def v_unit(b, ti):
                vt = fp.tile([128, VW], bf16, tag=f"v{b}_{ti}", name=f"v{b}_{ti}")
                chunks = ((0, 512), (512, VW - 512))
                pv = [psA.tile([128, 512], f32, tag="big", name=f"pv{ci}")
                      for ci in range(2)]
                for k in range(KT):
                    for ci, (c0, cw) in enumerate(chunks):
                        nc.tensor.matmul(pv[ci][:, 0:cw],
                                         xT[b][k][:, 2 + 128 * ti: 2 + 128 * (ti + 1)],
                                         wv_t[k][:, c0: c0 + cw],
                                         start=(k == 0), stop=(k == KT - 1))
                for ci, (c0, cw) in enumerate(chunks):
                    nc.vector.tensor_copy(vt[:, c0:c0 + cw], pv[ci][:, 0:cw])
                nc.vector.memset(vt[:, 64:VW:65], 1.0)
                vs[b][ti] = vt

            def attn_head(b, h):
                g, p0 = h // 2, 64 * (h % 2)
                kk, qq = kTt[b][g], qT[b][g]
                hps = psS.tile([128, 512], f32, tag="hps", name="hps")
                nc.tensor.matmul(hps[0:1, 0:TAUG], kk[p0:p0 + 64, 0:1], qq[p0:p0 + 64, :],
                                 start=True, stop=True, skip_group_check=True)
                ecr = sp.tile([1, TAUG], bf16, tag="ecr", name="ecr")
                nc.scalar.activation(ecr[:], hps[0:1, 0:TAUG], Exp, scale=SCALE)
                for s in range(2):
                    q0 = 2 + 128 * s
                    nc.tensor.matmul(clsp[b][:, 12 * s + h:12 * s + h + 1],
                                     kk[p0:p0 + 64, q0:q0 + 128],
                                     qq[p0:p0 + 64, 0:1], start=True, stop=True,
                                     skip_group_check=True)
                nc.tensor.matmul(clsp[b][0:2, 24 + 2 * h:24 + 2 * h + 2],
                                 kk[p0:p0 + 64, 0:2],
                                 qq[p0:p0 + 64, 0:2], start=True, stop=True,
                                 skip_group_check=True)
                for s in range(2):
                    q0 = 2 + 128 * s
                    pc = TAUG + 65 * s
                    pst = psP.tile([128, 128], f32, tag="pq", name="pst")
                    nc.tensor.matmul(pst[:], kk[p0:p0 + 64, q0:q0 + 128],
                                     qq[p0:p0 + 64, q0:q0 + 128], start=True, stop=True)
                    prob = sp.tile([128, 128], bf16, tag="prob", name="prob")
                    nc.scalar.activation(prob[:], pst[:], Exp, scale=SCALE)
                    nc.vector.tensor_mul(prob[:], prob[:], mask_t[s][:])
                    nc.tensor.matmul(hps[:, pc:pc + 65], prob[:],
                                     vs[b][s][:, 65 * h:65 * h + 65],
                                     start=True, stop=False, skip_group_check=True)
                    nc.tensor.matmul(hps[:, pc:pc + 65], ecr[:, q0:q0 + 128],
                                     vcr[b][0:1, 65 * h:65 * h + 65],
                                     start=False, stop=True, skip_group_check=True)
                    rec = sp.tile([128, 1], f32, tag="rec", name="rec")
                    nc.vector.reciprocal(rec[:], hps[:, pc + 64:pc + 65])
                    nc.scalar.activation(att_s[b][s][:, 64 * h:64 * h + 64],
                                         hps[:, pc:pc + 64], Copy, scale=rec[:, 0:1])

            def cls_unit(b):
                eccs = []
                for s in range(2):
                    E = sp.tile([128, HEADS], bf16, tag="ECC", name="E")
                    nc.scalar.activation(E[:], clsp[b][:, 12 * s:12 * (s + 1)], Exp, scale=SCALE)
                    eccs.append(E)
                clsA = psS.tile([128, 512], f32, tag="hps", name="clsA")
                clsB = psS.tile([128, 512], f32, tag="hps", name="clsB")
                for s in range(2):
                    nc.tensor.matmul(clsA[0:HEADS, :], eccs[s][:], vs[b][s][:, 0:512],
                                     start=(s == 0), stop=(s == 1), skip_group_check=True)
                    nc.tensor.matmul(clsB[0:HEADS, 0:VW - 512], eccs[s][:], vs[b][s][:, 512:VW],
                                     start=(s == 0), stop=(s == 1), skip_group_check=True)
                cls_sb = sp.tile([HEADS, VW], f32, tag="clssb", name="cls_sb")
                nc.vector.tensor_copy(cls_sb[:, 0:512], clsA[0:HEADS, :])
                nc.vector.tensor_copy(cls_sb[:, 512:VW], clsB[0:HEADS, 0:VW - 512])
                nc.sync.dma_start(cls_ext[b], cls_sb[:])
                ET1 = sp.tile([2, 2 * HEADS], bf16, tag="ET1", name="ET1")
                nc.scalar.activation(ET1[:], clsp[b][0:2, 24:24 + 2 * HEADS], Exp, scale=SCALE)
                t1A = psS.tile([128, 512], f32, tag="hps", name="t1A")
                t1B = psS.tile([128, 512], f32, tag="hps", name="t1B")
                nc.tensor.matmul(t1A[0:24, :], ET1[:], vcr[b][0:2, 0:512], start=True, stop=True,
                                 skip_group_check=True)
                nc.tensor.matmul(t1B[0:24, 0:VW - 512], ET1[:], vcr[b][0:2, 512:VW], start=True,
                                 stop=True, skip_group_check=True)
                t1_sb = sp.tile([24, VW], f32, tag="t1sb", name="t1_sb")
                nc.vector.tensor_copy(t1_sb[:, 0:512], t1A[0:24, :])
                nc.vector.tensor_copy(t1_sb[:, 512:VW], t1B[0:24, 0:VW - 512])
                nc.sync.dma_start(t1x_ext[b], t1_sb[:])

            def tr_unit(b, i):
                for s in range(2):
                    pt = psA.tile([128, 128], bf16, tag="big", name="pt")
                    nc.tensor.transpose(pt[:], att_s[b][s][:, 128 * i:128 * (i + 1)], ident[:])
                    nc.vector.tensor_copy(aT[b][i][:, 128 * s:128 * (s + 1)], pt[:])

            def proj_unit(b, ti):
                ot = fp.tile([128, DIM], bf16, tag=f"ot{b}_{ti}", name=f"ot{b}_{ti}")
                for c0, cw in ((0, 512), (512, 256)):
                    po = psA.tile([128, 512], f32, tag="big", name="po")
                    for i in range(KT):
                        nc.tensor.matmul(po[:, 0:cw], aT[b][i][:, 128 * ti:128 * (ti + 1)],
                                         wo_t[i][:, c0:c0 + cw],
                                         start=(i == 0), stop=(i == KT - 1))
                    nc.vector.tensor_add(ot[:, c0:c0 + cw], po[:, 0:cw],
                                         bias_full[:, c0:c0 + cw])
                nc.sync.dma_start(out_ext[b, 128 * ti:128 * (ti + 1), :], ot[:])

            # ---- phase 1: QKV(b0) dense ----
            for g in range(HEADS // 2):
                qk_unit(0, 2 * g, "q", g, qT)
                qk_unit(0, 2 * g + 1, "k", g, kTt)
            v_unit(0, 0)
            v_unit(0, 1)

            # ---- phase 2: attention(b0) interleaved with QKV(b1) ----
            b1_units = ([lambda g=g: qk_unit(1, 2 * g, "q", g, qT) for g in range(HEADS // 2)]
                        + [lambda g=g: qk_unit(1, 2 * g + 1, "k", g, kTt) for g in range(HEADS // 2)]
                        + [lambda ti=ti: v_unit(1, ti) for ti in range(2)])
            for h in range(HEADS):
                attn_head(0, h)
                u = b1_units[7 * h // 6:7 * (h + 1) // 6]
                for f in u:
                    f()
            for f in b1_units[14:]:
                f()

            # ---- phase 3: attention(b1) interleaved with cls/transpose/proj(b0) ----
            b0_tail = ([lambda: cls_unit(0)]
                       + [lambda i=i: tr_unit(0, i) for i in range(KT)]
                       + [lambda ti=ti: proj_unit(0, ti) for ti in range(2)])
            for h in range(HEADS):
                attn_head(1, h)
                u = b0_tail[9 * h // 12:9 * (h + 1) // 12]
                for f in u:
                    f()
            for f in b0_tail[9:]:
                f()

            # ---- phase 4: tail for b1 ----
            cls_unit(1)
            for i in range(KT):
                tr_unit(1, i)
            for ti in range(2):
                proj_unit(1, ti)

    nc.compile()
    return nc


def _get_nc():
    if "nc" not in _NC_CACHE:
        _NC_CACHE["nc"] = _build_nc()
    return _NC_CACHE["nc"]


def _make_masks(core):
    start = STARTS[core]
    masks = np.zeros((2, 128, 128), dtype=np.float32)
    for s in range(2):
        g = start + s * 128 + np.arange(128)
        real = g < 2049
        blk = (g - 2) // 16
        same = (blk[:, None] == blk[None, :]) & real[:, None] & real[None, :]
        masks[s] = same.astype(np.float32)
    return masks.astype(BF16)


def _make_in_maps(x, w_qkv, w_out, b_out):
    x = np.asarray(x, dtype=np.float32)
    w_qkv = np.asarray(w_qkv, dtype=np.float32)
    w_out_b = np.asarray(w_out, dtype=np.float32).astype(BF16)
    b_out = np.asarray(b_out, dtype=np.float32)

    # q/k weights as 12 column-slices [128, KT*128] (order q0,k0,q1,k1,...):
    # slice col-block k holds w_qkv[128k:128(k+1), cols].
    wqk = np.zeros((12, 128, KT * 128), dtype=np.float32)
    for g in range(HEADS // 2):
        for j, base in ((0, 0), (1, INNER)):
            cols = w_qkv[:, base + 128 * g: base + 128 * (g + 1)]  # [768, 128]
            wqk[2 * g + j] = cols.reshape(KT, 128, 128).transpose(1, 0, 2).reshape(128, KT * 128)
    wqk_b = wqk.astype(BF16)
    # v weights in 65-spaced layout, per dim-row tile [KT, 128, VW]
    wv = np.zeros((KT, 128, VW), dtype=np.float32)
    for h in range(HEADS):
        wv[:, :, 65 * h:65 * h + 64] = (
            w_qkv[:, 1536 + 64 * h:1536 + 64 * h + 64].reshape(KT, 128, 64))
    wv_b = wv.astype(BF16)

    w_v = w_qkv[:, 1536:]
    vcls = x[:, 0, :] @ w_v
    vtok1 = x[:, 1, :] @ w_v

    def v65(row768):
        out = np.zeros(VW, dtype=np.float32)
        for h in range(HEADS):
            out[65 * h:65 * h + 64] = row768[64 * h:64 * h + 64]
            out[65 * h + 64] = 1.0
        return out

    ident = np.eye(128, dtype=BF16)
    bias_tiled = np.tile(b_out.reshape(1, DIM), (128, 1)).astype(BF16)
    in_maps = []
    for c in range(NCORES):
        xa = np.zeros((B, TAUG, DIM), dtype=np.float32)
        xa[:, 0, :] = x[:, 0, :]
        if c == 0:
            xa[:, 1, :] = x[:, 1, :]
        L = ENDS[c] - STARTS[c]
        xa[:, 2:2 + L, :] = x[:, STARTS[c]:ENDS[c], :]
        xaT = xa.transpose(0, 2, 1).reshape(B, KT, 128, TAUG).astype(BF16)
        vcr = np.zeros((B, 2, VW), dtype=np.float32)
        for b in range(B):
            vcr[b, 0] = v65(vcls[b])
            vcr[b, 1] = v65(vtok1[b] if c == 0 else np.zeros(INNER, np.float32))
        in_maps.append({
            "vc_rep": vcr.astype(BF16),
            "xaT": xaT,
            "wqk": wqk_b,
            "wv": wv_b,
            "w_out": w_out_b,
            "b_out": bias_tiled,
            "masks": _make_masks(c),
            "ident": ident,
        })
    return in_maps


def kernel(x, w_qkv, w_out, b_out):
    w_out_f = np.asarray(w_out, dtype=np.float32)
    b_out_f = np.asarray(b_out, dtype=np.float32)
    in_maps = _make_in_maps(x, w_qkv, w_out, b_out)

    from concourse.bass_utils import run_bass_kernel_spmd

    nc = _get_nc()
    res = run_bass_kernel_spmd(nc, in_maps, core_ids=list(range(NCORES))).results

    out = np.empty((B, N, DIM), dtype=np.float32)
    for c in range(NCORES):
        L = ENDS[c] - STARTS[c]
        out[:, STARTS[c]:ENDS[c], :] = res[c]["out_tokens"][:, :L, :]

    for b in range(B):
        acc = np.zeros((HEADS, VW), dtype=np.float64)
        for c in range(NCORES):
            acc += res[c]["cls_part"][b].astype(np.float64)
            acc[:, 64::65] -= 256 - (ENDS[c] - STARTS[c])
        t1x = res[0]["t1x_part"][b].astype(np.float64)
        acc += t1x[0::2]
        cls_flat = np.empty(INNER, dtype=np.float64)
        t1_flat = np.empty(INNER, dtype=np.float64)
        for h in range(HEADS):
            cls_flat[64 * h:64 * h + 64] = acc[h, 65 * h:65 * h + 64] / acc[h, 65 * h + 64]
            t1_flat[64 * h:64 * h + 64] = (t1x[2 * h + 1, 65 * h:65 * h + 64]
                                           / t1x[2 * h + 1, 65 * h + 64])
        out[b, 0, :] = (cls_flat @ w_out_f + b_out_f).astype(np.float32)
        out[b, 1, :] = (t1_flat @ w_out_f + b_out_f).astype(np.float32)
    return out

